# revision 1
# baseline (speedup 1.0000x reference)
"""Trainium2 Bass kernel for CasAttention2D.

Math (reference):
    kh  = k @ Wk;  v = kh @ Wv;  qh = q @ Wq
    ph  = relu(pos @ P1 + pb1) @ P2 + pb2
    s   = kh - qh[:,:,None,:] + ph
    a   = relu(s @ A1 + ab1) @ A2 + ab2
    a   = where(mask==0, -1e9, a); attn = softmax(a, axis=K)
    out = ((v + ph) * attn).sum(K) @ Wo + bo

Device-side reformulation (per token-row r = (token, k)):
    kmq = k - q_broadcast                       (host fold)
    vph = kmq @ (Wk Wv) + relu(pos@P1+pb1) @ P2 + q @ (Wk Wv) + pb2
    s1  = kmq @ (Wk A1) + relu(pos@P1+pb1) @ (P2 A1) + (ab1 + pb2@A1)
    a2  = relu(s1) @ A2 + madd                  (ab2 dropped: softmax-invariant)
    num = exp(a2);  den = segsum_K(num)
    out_f = Wo^T (segsum_K(vph*num) / den) + bo

Everything on-device runs feature-major: SBUF tiles are [feature, row].
The host transposes inputs once and transposes the output back.
"""

import numpy as np
from contextlib import ExitStack

import sys

for _p in ("/root/.axon_site/_ro/trn_rl_repo", "/root/.axon_site/_ro/pypackages",
           "/opt/trn_rl_repo", "/opt/pypackages"):
    if _p not in sys.path:
        sys.path.append(_p)

import concourse.bass as bass
import concourse.tile as tile
from concourse import mybir
from concourse.bass_utils import run_bass_kernel_spmd

# problem dims (hardcoded per contract)
B, N, K, D = 4, 4096, 16, 128
H = D // 8
NCORES = 8
T_TOTAL = B * N                 # 16384 tokens
T_CORE = T_TOTAL // NCORES      # 2048 tokens per core
R_CORE = T_CORE * K             # 32768 k-rows per core
CHUNK = 512                     # k-rows per chunk (32 tokens)
TOK_CHUNK = CHUNK // K          # 32 tokens per chunk
NCHUNK = R_CORE // CHUNK        # 64
GRP = 8                         # chunks per output group (256 tokens)
TOK_GRP = GRP * TOK_CHUNK       # 256

F32 = mybir.dt.float32
F32R = mybir.dt.float32r
AF = mybir.ActivationFunctionType
ALU = mybir.AluOpType


def _legalize_waits(nc):
    """This walrus build encodes at most ONE sync-wait per instruction.
    Split multi-wait instructions into single-wait same-engine NoOps."""
    cnt = 0
    for fn in nc.m.functions:
        for blk in fn.blocks:
            bb = blk.bb if hasattr(blk, "bb") else blk
            insts = bb.instructions
            new_list = []
            for inst in insts:
                si = inst.sync_info
                waits = list(si.on_wait) if (si and si.on_wait) else []
                if len(waits) > 1:
                    for w in waits[:-1]:
                        cnt += 1
                        nop = mybir.InstNoOp(
                            name=f"WSPLIT-{cnt}-{inst.name}",
                            sync_info=mybir.SyncInfo(on_wait=[w], on_update=[]),
                        )
                        nop.engine = inst.engine
                        new_list.append(nop)
                    si.on_wait = [waits[-1]]
                new_list.append(inst)
            del insts[:]
            for x in new_list:
                insts.append(x)
    return cnt


def _build_program(uadd_chunks):
    """Build the SPMD Bass program. uadd_chunks: set of chunk indices that
    need the all-masked-token uniform-leak correction."""
    nc = bass.Bass()

    # per-core DRAM inputs (feature-major)
    kf = nc.dram_tensor("kf", [D, R_CORE], F32R, kind="ExternalInput")
    posf = nc.dram_tensor("posf", [4, R_CORE], F32R, kind="ExternalInput")
    qf = nc.dram_tensor("qf", [D, T_CORE], F32R, kind="ExternalInput")
    madd = nc.dram_tensor("madd", [1, R_CORE], F32R, kind="ExternalInput")
    uadd = nc.dram_tensor("uadd", [1, R_CORE], F32, kind="ExternalInput")

    w_kv = nc.dram_tensor("w_kv", [D, D], F32R, kind="ExternalInput")
    w_ka = nc.dram_tensor("w_ka", [D, H], F32R, kind="ExternalInput")
    w_p1 = nc.dram_tensor("w_p1", [4, H], F32R, kind="ExternalInput")
    w_p2 = nc.dram_tensor("w_p2", [H, D], F32R, kind="ExternalInput")
    w_p2a = nc.dram_tensor("w_p2a", [H, H], F32R, kind="ExternalInput")
    w_a2 = nc.dram_tensor("w_a2", [H, D], F32R, kind="ExternalInput")
    w_o = nc.dram_tensor("w_o", [D, D], F32, kind="ExternalInput")
    w_ones = nc.dram_tensor("w_ones", [1, D], F32R, kind="ExternalInput")
    w_nqa = nc.dram_tensor("w_nqa", [D, H], F32R, kind="ExternalInput")
    b_p1 = nc.dram_tensor("b_p1", [H, 1], F32, kind="ExternalInput")
    b_s1 = nc.dram_tensor("b_s1", [H, 1], F32, kind="ExternalInput")
    b_p2 = nc.dram_tensor("b_p2", [D, 1], F32, kind="ExternalInput")
    b_o = nc.dram_tensor("b_o", [D, 1], F32, kind="ExternalInput")

    out_f = nc.dram_tensor("out_f", [D, T_CORE], F32, kind="ExternalOutput")

    with ExitStack() as ctx:
        tc = ctx.enter_context(tile.TileContext(nc))
        consts = ctx.enter_context(tc.tile_pool(name="consts", bufs=1))
        kpool = ctx.enter_context(tc.tile_pool(name="kpool", bufs=3))
        spool = ctx.enter_context(tc.tile_pool(name="spool", bufs=3))
        vpool = ctx.enter_context(tc.tile_pool(name="vpool", bufs=3))
        dpool = ctx.enter_context(tc.tile_pool(name="dpool", bufs=4))
        gpool = ctx.enter_context(tc.tile_pool(name="gpool", bufs=2))
        ps_misc = ctx.enter_context(tc.tile_pool(name="ps_misc", bufs=1, space="PSUM"))
        ps_p1 = ctx.enter_context(tc.tile_pool(name="ps_p1", bufs=2, space="PSUM"))
        ps_s1 = ctx.enter_context(tc.tile_pool(name="ps_s1", bufs=2, space="PSUM"))
        ps_vph = ctx.enter_context(tc.tile_pool(name="ps_vph", bufs=2, space="PSUM"))
        ps_a2 = ctx.enter_context(tc.tile_pool(name="ps_a2", bufs=1, space="PSUM"))

        # load weights/biases once (distinct tags: one resident slot each)
        def wtile(dram, shape, dt=F32R):
            t = consts.tile(shape, dt, tag=f"w_{dram.name}")
            nc.sync.dma_start(out=t, in_=dram[:])
            return t

        Wkv = wtile(w_kv, [D, D])
        Wka = wtile(w_ka, [D, H])
        P1 = wtile(w_p1, [4, H])
        P2 = wtile(w_p2, [H, D])
        P2a = wtile(w_p2a, [H, H])
        A2 = wtile(w_a2, [H, D])
        Wo = wtile(w_o, [D, D], F32)
        Ones1 = wtile(w_ones, [1, D])
        NQa = wtile(w_nqa, [D, H])
        Bp1 = wtile(b_p1, [H, 1], F32)
        Bs1 = wtile(b_s1, [H, 1], F32)
        Bp2 = wtile(b_p2, [D, 1], F32)
        Bo = wtile(b_o, [D, 1], F32)

        for c in range(NCHUNK):
            g = c // GRP
            ci = c % GRP
            r0 = c * CHUNK
            t0 = ci * TOK_CHUNK  # token offset within group

            if ci == 0:
                # per-group q tile + qv = Wkv^T q (+ pb2) in SBUF
                qt = gpool.tile([D, TOK_GRP], F32R, tag="qt")
                nc.sync.dma_start(out=qt, in_=qf[:, g * TOK_GRP:(g + 1) * TOK_GRP])
                # per-group output accumulator
                xsup = gpool.tile([D, TOK_GRP], F32, tag="xsup")

            kt = kpool.tile([D, CHUNK], F32R, tag="kmq")
            nc.sync.dma_start(out=kt, in_=kf[:, r0:r0 + CHUNK])
            post = kpool.tile([4, CHUNK], F32R, tag="pos")
            nc.sync.dma_start(out=post, in_=posf[:, r0:r0 + CHUNK])
            maddt = kpool.tile([1, CHUNK], F32R, tag="madd")
            nc.sync.dma_start(out=maddt, in_=madd[:, r0:r0 + CHUNK])

            # pos MLP first layer
            p1_ps = ps_p1.tile([H, CHUNK], F32, tag="p1")
            nc.tensor.matmul(p1_ps[:], P1[:], post[:], start=True, stop=True)
            r1 = spool.tile([H, CHUNK], F32R, tag="r1")
            nc.vector.tensor_scalar(out=r1[:], in0=p1_ps[:], scalar1=Bp1[:],
                                    scalar2=0.0, op0=ALU.add, op1=ALU.max)

            # attention-MLP hidden pre-act
            s1_ps = ps_s1.tile([H, CHUNK], F32, tag="s1")
            nc.tensor.matmul(s1_ps[:], Wka[:], kt[:], start=True, stop=False)
            nc.tensor.matmul(s1_ps[:], P2a[:], r1[:], start=False, stop=False)
            qb = qt[:, t0:t0 + TOK_CHUNK].unsqueeze(2).broadcast_to(
                (D, TOK_CHUNK, K))
            nc.tensor.matmul(s1_ps[:], NQa[:], qb, start=False, stop=True)
            a1 = spool.tile([H, CHUNK], F32R, tag="a1")
            nc.vector.tensor_scalar(out=a1[:], in0=s1_ps[:], scalar1=Bs1[:],
                                    scalar2=0.0, op0=ALU.add, op1=ALU.max)

            # values v+ph (PSUM-accumulated)
            vph_ps = ps_vph.tile([D, CHUNK], F32, tag="vph")
            nc.tensor.matmul(vph_ps[:], Wkv[:], kt[:], start=True, stop=False)
            nc.tensor.matmul(vph_ps[:], P2[:], r1[:], start=False, stop=True)

            # logits + additive mask (PSUM-accumulated)
            a2_ps = ps_a2.tile([D, CHUNK], F32, tag="a2")
            nc.tensor.matmul(a2_ps[:], A2[:], a1[:], start=True, stop=False)
            nc.tensor.matmul(a2_ps[:], Ones1[:], maddt[:], start=False, stop=True)

            num = vpool.tile([D, CHUNK], F32, tag="num")
            nc.scalar.activation(num[:], a2_ps[:], AF.Exp)
            if c in uadd_chunks:
                uaddt = kpool.tile([1, CHUNK], F32, tag="uadd")
                nc.sync.dma_start(out=uaddt, in_=uadd[:, r0:r0 + CHUNK])
                ub = uaddt[:].partition_broadcast(D).rearrange("p q f -> p (q f)")
                nc.vector.tensor_tensor(out=num[:], in0=num[:], in1=ub,
                                        op=ALU.add)

            den = dpool.tile([D, TOK_CHUNK], F32, tag="den")
            nc.vector.tensor_reduce(out=den[:], in_=num[:].rearrange(
                "p (a b) -> p a b", b=K), axis=mybir.AxisListType.X, op=ALU.add)
            rec = dpool.tile([D, TOK_CHUNK], F32, tag="rec")
            nc.vector.reciprocal(out=rec[:], in_=den[:])

            # vph to SBUF with per-feature pb2 bias
            vph = vpool.tile([D, CHUNK], F32, tag="vphsb")
            nc.vector.tensor_scalar(out=vph[:], in0=vph_ps[:], scalar1=Bp2[:],
                                    scalar2=None, op0=ALU.add)

            y = vpool.tile([D, CHUNK], F32, tag="y")
            nc.vector.tensor_tensor(out=y[:], in0=vph[:], in1=num[:],
                                    op=ALU.mult)
            ynum = dpool.tile([D, TOK_CHUNK], F32, tag="ynum")
            nc.vector.tensor_reduce(out=ynum[:], in_=y[:].rearrange(
                "p (a b) -> p a b", b=K), axis=mybir.AxisListType.X, op=ALU.add)
            nc.vector.tensor_tensor(out=xsup[:, t0:t0 + TOK_CHUNK],
                                    in0=ynum[:], in1=rec[:], op=ALU.mult)

            if ci == GRP - 1:
                wo_ps = ps_misc.tile([D, TOK_GRP], F32, tag="misc")
                nc.tensor.matmul(wo_ps[:], Wo[:], xsup[:],
                                 start=True, stop=True)
                outt = gpool.tile([D, TOK_GRP], F32, tag="outt")
                nc.scalar.activation(outt[:], wo_ps[:], AF.Identity, bias=Bo[:])
                nc.sync.dma_start(out=out_f[:, g * TOK_GRP:(g + 1) * TOK_GRP],
                                  in_=outt[:])

    _legalize_waits(nc)
    return nc


_CACHE = {}


def kernel(q, k, pos, mask, Wq, Wk, Wv, P1, pb1, P2, pb2,
           A1, ab1, A2, ab2, Wo, bo):
    q = np.asarray(q, np.float32)
    k = np.asarray(k, np.float32)
    pos = np.asarray(pos, np.float32)
    mask_np = np.asarray(mask)
    Wq, Wk, Wv = (np.asarray(x, np.float32) for x in (Wq, Wk, Wv))
    P1, pb1, P2, pb2 = (np.asarray(x, np.float32) for x in (P1, pb1, P2, pb2))
    A1, ab1, A2, ab2 = (np.asarray(x, np.float32) for x in (A1, ab1, A2, ab2))
    Wo, bo = np.asarray(Wo, np.float32), np.asarray(bo, np.float32)

    # ---- host-side input prep (layout + weight folding) ----
    kT = np.ascontiguousarray(k.reshape(T_TOTAL * K, D).T)     # [D, R]
    posT = np.ascontiguousarray(pos.reshape(T_TOTAL * K, 4).T)  # [4, R]
    qT = np.ascontiguousarray(q.reshape(T_TOTAL, D).T)  # [D, T]
    m = mask_np.reshape(T_TOTAL, K) != 0
    maddv = np.where(m, np.float32(0), np.float32(-1e9)).reshape(1, -1)
    all_masked = ~m.any(axis=1)                         # [T]
    uaddv = np.repeat(all_masked.astype(np.float32), K).reshape(1, -1)

    w_kv = np.ascontiguousarray(Wk @ Wv)
    w_ka = np.ascontiguousarray(Wk @ A1)
    w_p2a = np.ascontiguousarray(P2 @ A1)
    w_nqa = np.ascontiguousarray(-(Wq @ A1))
    b_s1 = (ab1 + pb2 @ A1).reshape(H, 1)
    w_ones = np.ones((1, D), np.float32)

    # which chunks need the uniform-leak correction (per core -> global union;
    # SPMD shares one program, so apply the union of chunk indices)
    uadd_chunks = set()
    if all_masked.any():
        idx = np.nonzero(all_masked)[0]
        for t in idx:
            core = t // T_CORE
            local_tok = t - core * T_CORE
            uadd_chunks.add(local_tok // TOK_CHUNK)

    key = ("v2", tuple(sorted(uadd_chunks)))
    if key not in _CACHE:
        _CACHE[key] = _build_program(uadd_chunks)
    nc = _CACHE[key]

    shared = {
        "w_kv": w_kv, "w_ka": w_ka, "w_p1": P1, "w_p2": np.ascontiguousarray(P2),
        "w_p2a": w_p2a, "w_a2": np.ascontiguousarray(A2),
        "w_o": np.ascontiguousarray(Wo), "w_ones": w_ones, "w_nqa": w_nqa,
        "b_p1": pb1.reshape(H, 1), "b_s1": b_s1,
        "b_p2": pb2.reshape(D, 1), "b_o": bo.reshape(D, 1),
    }
    in_maps = []
    for c in range(NCORES):
        rs, re = c * R_CORE, (c + 1) * R_CORE
        ts, te = c * T_CORE, (c + 1) * T_CORE
        im = dict(shared)
        im["kf"] = np.ascontiguousarray(kT[:, rs:re])
        im["posf"] = np.ascontiguousarray(posT[:, rs:re])
        im["qf"] = np.ascontiguousarray(qT[:, ts:te])
        im["madd"] = np.ascontiguousarray(maddv[:, rs:re])
        im["uadd"] = np.ascontiguousarray(uaddv[:, rs:re])
        in_maps.append(im)

    res = run_bass_kernel_spmd(nc, in_maps, core_ids=list(range(NCORES)))
    kernel._last_results = res
    out = np.concatenate([res.results[c]["out_f"] for c in range(NCORES)],
                         axis=1)                        # [D, T]
    return np.ascontiguousarray(out.T).reshape(B, N, D).astype(np.float32)



# revision 4
# speedup vs baseline: 7.0927x; 7.0927x over previous
"""Trainium2 Bass kernel for CasAttention2D — v3.

Math (reference):
    kh  = k @ Wk;  v = kh @ Wv;  qh = q @ Wq
    ph  = relu(pos @ P1 + pb1) @ P2 + pb2
    s   = kh - qh[:,:,None,:] + ph
    a   = relu(s @ A1 + ab1) @ A2 + ab2
    a   = where(mask==0, -1e9, a); attn = softmax(a, axis=K)
    out = ((v + ph) * attn).sum(K) @ Wo + bo

Device-side reformulation (feature-major; per k-row r = (token, k)):
    host: r1  = relu(pos@P1 + pb1)                         [R, H]
          qab = -q@(Wq A1) + (ab1 + pb2@A1), repeated K    [R, H]
          rq  = [r1; qab]^T  (bf16)                        [2H, R]
          mc  = (1 - mask)   (bf16)                        [R]
    dev:  s1  = (Wk A1)^T kt + [P2 A1; I]^T rq             [H, cols]
          a1  = relu(s1)   (4 chunks stacked at PE col-tiles 0/32/64/96,
                            one Act op per stack)
          a2  = [A2; -1e9]^T [a1; mc]                      [D, cols]
          num = exp(a2)            (masked rows underflow to exactly 0)
          vph = (Wk Wv)^T kt + P2^T rq[:H]
          den = treeadd_K(num)          [4 TT-adds on GPSIMD]
          ynum= segreduce_K(vph * num)  [DVE]
          xsup= ynum * (1/den);  out = Wo^T xsup + (bo + pb2@Wo)
          (group tails deferred 3 chunks so the in-order PE queue
           never parks on the DVE/GPSIMD tail chain)
"""

import numpy as np
from contextlib import ExitStack

import sys

for _p in ("/root/.axon_site/_ro/trn_rl_repo", "/root/.axon_site/_ro/pypackages",
           "/opt/trn_rl_repo", "/opt/pypackages"):
    if _p not in sys.path:
        sys.path.append(_p)

import concourse.bass as bass
import concourse.tile as tile
from concourse import mybir
from concourse.bass_utils import run_bass_kernel_spmd

# problem dims (hardcoded per contract)
B, N, K, D = 4, 4096, 16, 128
H = D // 8
NCORES = 8
T_TOTAL = B * N                 # 16384 tokens
T_CORE = T_TOTAL // NCORES      # 2048 tokens per core
R_CORE = T_CORE * K             # 32768 k-rows per core
CHUNK = 512                     # k-rows per chunk (32 tokens)
TOK_CHUNK = CHUNK // K          # 32 tokens per chunk
NCHUNK = R_CORE // CHUNK        # 64
STACK = 4                       # chunks per relu stack (slots at 0/32/64/96)
SLOT = 32                       # partition stride between stack slots
NSTACK = NCHUNK // STACK        # 16
GRP = 8                         # chunks per output group (256 tokens)
TOK_GRP = GRP * TOK_CHUNK       # 256
DMAB = 4                        # chunks per input DMA batch

F32 = mybir.dt.float32
F32R = mybir.dt.float32r
BF16 = mybir.dt.bfloat16
AF = mybir.ActivationFunctionType
ALU = mybir.AluOpType

def _legalize_waits(nc):
    """This walrus build encodes at most ONE sync-wait per instruction.
    Split multi-wait instructions into single-wait same-engine NoOps."""
    cnt = 0
    for fn in nc.m.functions:
        for blk in fn.blocks:
            bb = blk.bb if hasattr(blk, "bb") else blk
            insts = bb.instructions
            new_list = []
            for inst in insts:
                si = inst.sync_info
                waits = list(si.on_wait) if (si and si.on_wait) else []
                if len(waits) > 1:
                    for w in waits[:-1]:
                        cnt += 1
                        nop = mybir.InstNoOp(
                            name=f"WSPLIT-{cnt}-{inst.name}",
                            sync_info=mybir.SyncInfo(on_wait=[w], on_update=[]),
                        )
                        nop.engine = inst.engine
                        new_list.append(nop)
                    si.on_wait = [waits[-1]]
                new_list.append(inst)
            del insts[:]
            for x in new_list:
                insts.append(x)
    return cnt


def _build_program(uadd_chunks=(), reps=1):
    nc = bass.Bass()

    kf = nc.dram_tensor("kf", [D, R_CORE], BF16, kind="ExternalInput")
    rqf = nc.dram_tensor("rqf", [2 * H, R_CORE], BF16, kind="ExternalInput")
    mrows = nc.dram_tensor("mrows", [NCHUNK, CHUNK], BF16, kind="ExternalInput")
    uadd = nc.dram_tensor("uadd", [1, R_CORE], BF16, kind="ExternalInput")

    w_kv = nc.dram_tensor("w_kv", [D, D], BF16, kind="ExternalInput")
    w_ka = nc.dram_tensor("w_ka", [D, H], BF16, kind="ExternalInput")
    w_p2ai = nc.dram_tensor("w_p2ai", [2 * H, H], BF16, kind="ExternalInput")
    w_p2 = nc.dram_tensor("w_p2", [H, D], BF16, kind="ExternalInput")
    # A2m replicated at partitions 0/32/64/96 (PE tile positions)
    w_a2m = nc.dram_tensor("w_a2m", [D, D], BF16, kind="ExternalInput")
    w_o = nc.dram_tensor("w_o", [D, D], F32R, kind="ExternalInput")
    b_o = nc.dram_tensor("b_o", [D, 1], F32, kind="ExternalInput")

    out_f = nc.dram_tensor("out_f", [D, T_CORE], F32, kind="ExternalOutput")

    with ExitStack() as ctx:
        tc = ctx.enter_context(tile.TileContext(nc))
        consts = ctx.enter_context(tc.tile_pool(name="consts", bufs=1))
        kpool = ctx.enter_context(tc.tile_pool(name="kpool", bufs=4))
        rqpool = ctx.enter_context(tc.tile_pool(name="rqpool", bufs=4))
        apool = ctx.enter_context(tc.tile_pool(name="apool", bufs=3))
        npool = ctx.enter_context(tc.tile_pool(name="npool", bufs=3))
        ypool = ctx.enter_context(tc.tile_pool(name="ypool", bufs=3))
        dpool = ctx.enter_context(tc.tile_pool(name="dpool", bufs=2))
        rpool = ctx.enter_context(tc.tile_pool(name="rpool", bufs=2))
        gpool = ctx.enter_context(tc.tile_pool(name="gpool", bufs=2))
        ps_s1 = ctx.enter_context(tc.tile_pool(name="ps_s1", bufs=2, space="PSUM"))
        ps_vph = ctx.enter_context(tc.tile_pool(name="ps_vph", bufs=3, space="PSUM"))
        ps_a2 = ctx.enter_context(tc.tile_pool(name="ps_a2", bufs=2, space="PSUM"))
        ps_wo = ctx.enter_context(tc.tile_pool(name="ps_wo", bufs=1, space="PSUM"))

        def wtile(dram, shape, dt):
            t = consts.tile(shape, dt, tag=f"w_{dram.name}")
            # Act HWDGE ring: overlaps with the big input DMAs on the SP ring
            nc.scalar.dma_start(out=t, in_=dram[:])
            return t

        Wkv = wtile(w_kv, [D, D], BF16)
        Wka = wtile(w_ka, [D, H], BF16)
        P2aI = wtile(w_p2ai, [2 * H, H], BF16)
        P2 = wtile(w_p2, [H, D], BF16)
        A2m = wtile(w_a2m, [D, D], BF16)
        Wo = wtile(w_o, [D, D], F32R)
        Bo = wtile(b_o, [D, 1], F32)

        kts = {}
        rqs = {}

        def phase1(c):
            """DMA + s1 matmuls for chunk c into the stack PSUM."""
            bi = c // DMAB
            if bi not in kts:
                kt = kpool.tile([D, DMAB * CHUNK], BF16, tag="kt")
                nc.sync.dma_start(
                    out=kt, in_=kf[:, bi * DMAB * CHUNK:(bi + 1) * DMAB * CHUNK])
                rqt = rqpool.tile([2 * H, DMAB * CHUNK], BF16, tag="rq")
                nc.sync.dma_start(
                    out=rqt, in_=rqf[:, bi * DMAB * CHUNK:(bi + 1) * DMAB * CHUNK])
                kts[bi] = kt
                rqs[bi] = rqt
            return kts[bi], rqs[bi]

        group_tiles = {}

        def emit_tail(g, xsup, den_g, ynum_g):
            rec = rpool.tile([D, TOK_GRP], F32, tag="rec")
            nc.vector.reciprocal(out=rec[:], in_=den_g[:])
            nc.vector.tensor_tensor(out=xsup[:], in0=ynum_g[:],
                                    in1=rec[:], op=ALU.mult)
            wo_ps = ps_wo.tile([D, TOK_GRP], F32, tag="wo")
            nc.tensor.matmul(wo_ps[:], Wo[:], xsup[:], start=True, stop=True)
            outt = gpool.tile([D, TOK_GRP], F32, tag="outt")
            nc.scalar.activation(outt[:], wo_ps[:], AF.Identity, bias=Bo[:])
            nc.scalar.dma_start(out=out_f[:, g * TOK_GRP:(g + 1) * TOK_GRP],
                              in_=outt[:])

        for rep in range(reps):
          kts.clear()
          rqs.clear()
          for s in range(NSTACK):
            cs = list(range(s * STACK, (s + 1) * STACK))

            # ---- phase 1: s1 matmuls into the stacked PSUM tile ----
            s1_ps = ps_s1.tile([D, CHUNK], F32, tag="s1stack")
            for ci, c in enumerate(cs):
                kt, rqt = phase1(c)
                off = (c % DMAB) * CHUNK
                o = ci * SLOT
                nc.tensor.matmul(s1_ps[o:o + H, :], Wka[:],
                                 kt[:, off:off + CHUNK], start=True, stop=False,
                                 tile_position=(0, o))
                nc.tensor.matmul(s1_ps[o:o + H, :], P2aI[:],
                                 rqt[:, off:off + CHUNK], start=False, stop=True,
                                 tile_position=(0, o))

            # ---- stacked relu + mask rows ----
            a1 = apool.tile([D, CHUNK], BF16, tag="a1stack")
            nc.scalar.activation(a1[:], s1_ps[:], AF.Relu)
            # mask rows at partitions o+H (o = 0/32/64/96)
            mrow_dst = a1[:].rearrange("(a b) f -> a b f", b=SLOT)[:, H:H + 1, :] \
                .rearrange("a b f -> (a b) f")
            nc.scalar.dma_start(out=mrow_dst, in_=mrows[cs[0]:cs[0] + STACK, :])

            # ---- phase 2 per chunk ----
            for ci, c in enumerate(cs):
                kt, rqt = kts[c // DMAB], rqs[c // DMAB]
                off = (c % DMAB) * CHUNK
                g = c // GRP
                t0 = (c % GRP) * TOK_CHUNK

                if c % GRP == 0:
                    xsup = gpool.tile([D, TOK_GRP], F32R, tag="xsup")

                vph_ps = ps_vph.tile([D, CHUNK], F32, tag="vph")
                nc.tensor.matmul(vph_ps[:], Wkv[:], kt[:, off:off + CHUNK],
                                 start=True, stop=False)
                nc.tensor.matmul(vph_ps[:], P2[:],
                                 rqt[0:H, off:off + CHUNK], start=False, stop=True)

                a2_ps = ps_a2.tile([D, CHUNK], F32, tag="a2")
                o = ci * SLOT
                nc.tensor.matmul(a2_ps[:], A2m[o:o + H + 1, :],
                                 a1[o:o + H + 1, :], start=True, stop=True,
                                 tile_position=(o, 0))

                num = npool.tile([D, CHUNK], BF16, tag="num")
                nc.scalar.activation(num[:], a2_ps[:], AF.Exp)
                if c in uadd_chunks:
                    uat = npool.tile([1, CHUNK], BF16, tag="uadd")
                    nc.sync.dma_start(out=uat, in_=uadd[:, c * CHUNK:(c + 1) * CHUNK])
                    ub = uat[:].partition_broadcast(D).rearrange("p q f -> p (q f)")
                    nc.vector.tensor_tensor(out=num[:], in0=num[:], in1=ub,
                                            op=ALU.add)

                if c % GRP == 0:
                    den_g = gpool.tile([D, TOK_GRP], F32, tag="den_g")
                    ynum_g = gpool.tile([D, TOK_GRP], F32, tag="ynum_g")

                y = ypool.tile([D, CHUNK], BF16, tag="y")
                nc.vector.tensor_tensor(out=y[:], in0=vph_ps[:], in1=num[:],
                                        op=ALU.mult)

                # den: 4-level binary tree on GPSIMD (t-major, K halves)
                num3 = num[:].rearrange("p (a b) -> p a b", b=K)
                t1 = dpool.tile([D, CHUNK // 2], BF16, tag="dt1")
                t13 = t1[:].rearrange("p (a b) -> p a b", b=K // 2)
                nc.gpsimd.tensor_tensor(out=t13, in0=num3[:, :, 0:8],
                                        in1=num3[:, :, 8:16], op=ALU.add)
                t2 = dpool.tile([D, CHUNK // 4], BF16, tag="dt2")
                t23 = t2[:].rearrange("p (a b) -> p a b", b=K // 4)
                nc.gpsimd.tensor_tensor(out=t23, in0=t13[:, :, 0:4],
                                        in1=t13[:, :, 4:8], op=ALU.add)
                t3 = dpool.tile([D, CHUNK // 8], BF16, tag="dt3")
                t33 = t3[:].rearrange("p (a b) -> p a b", b=K // 8)
                nc.gpsimd.tensor_tensor(out=t33, in0=t23[:, :, 0:2],
                                        in1=t23[:, :, 2:4], op=ALU.add)
                dg3 = den_g[:, t0:t0 + TOK_CHUNK] \
                    .rearrange("p (a b) -> p a b", b=1)
                nc.gpsimd.tensor_tensor(out=dg3, in0=t33[:, :, 0:1],
                                        in1=t33[:, :, 1:2], op=ALU.add)

                # ynum: segmented reduce on DVE -> fp32 group slice
                y3 = y[:].rearrange("p (a b) -> p a b", b=K)
                nc.vector.tensor_reduce(out=ynum_g[:, t0:t0 + TOK_CHUNK],
                                        in_=y3, axis=mybir.AxisListType.X,
                                        op=ALU.add)

                if c % GRP == GRP - 1:
                    group_tiles[g] = (xsup, den_g, ynum_g)
                # deferred group tail: emit 3 chunks into the next group so
                # the in-order PE queue never waits on the Pool/DVE tail chain
                gprev = g - 1
                if c % GRP == 3 and gprev in group_tiles:
                    emit_tail(gprev, *group_tiles.pop(gprev))

          while group_tiles:
            g = min(group_tiles)
            emit_tail(g, *group_tiles.pop(g))

    _legalize_waits(nc)
    return nc


_CACHE = {}


def kernel(q, k, pos, mask, Wq, Wk, Wv, P1, pb1, P2, pb2,
           A1, ab1, A2, ab2, Wo, bo):
    import ml_dtypes
    bf16 = ml_dtypes.bfloat16

    q = np.asarray(q, np.float32)
    k = np.asarray(k, np.float32)
    pos = np.asarray(pos, np.float32)
    mask_np = np.asarray(mask)
    Wq, Wk, Wv = (np.asarray(x, np.float32) for x in (Wq, Wk, Wv))
    P1, pb1, P2, pb2 = (np.asarray(x, np.float32) for x in (P1, pb1, P2, pb2))
    A1, ab1, A2, ab2 = (np.asarray(x, np.float32) for x in (A1, ab1, A2, ab2))
    Wo, bo = np.asarray(Wo, np.float32), np.asarray(bo, np.float32)

    # ---- host-side input prep (layout + weight/bias folding) ----
    R = T_TOTAL * K
    kT = np.ascontiguousarray(k.reshape(R, D).T)                      # [D, R]
    r1 = np.maximum(pos.reshape(R, 4) @ P1 + pb1, 0.0)                # [R, H]
    qab = q.reshape(T_TOTAL, D) @ (-(Wq @ A1)) + (ab1 + pb2 @ A1)     # [T, H]
    qab_r = np.repeat(qab, K, axis=0)                                 # [R, H]
    rqf = np.ascontiguousarray(
        np.concatenate([r1, qab_r], axis=1).T.astype(bf16))           # [2H, R]

    m = mask_np.reshape(T_TOTAL, K) != 0
    mc = (~m).astype(np.float32).reshape(-1)                          # [R]
    all_masked = ~m.any(axis=1)

    w_kv = np.ascontiguousarray((Wk @ Wv).astype(bf16))
    w_ka = np.ascontiguousarray((Wk @ A1).astype(bf16))
    w_p2ai = np.ascontiguousarray(
        np.concatenate([P2 @ A1, np.eye(H, dtype=np.float32)], axis=0)
        .astype(bf16))                                                # [2H, H]
    w_p2 = np.ascontiguousarray(P2.astype(bf16))
    a2m_blk = np.concatenate([A2, np.full((1, D), -1e9, np.float32)], axis=0)
    w_a2m = np.zeros((D, D), np.float32)                              # [D, D]
    for o in range(0, D, SLOT):
        w_a2m[o:o + H + 1] = a2m_blk
    w_a2m = np.ascontiguousarray(w_a2m.astype(bf16))
    b_o = (pb2 @ Wo + bo).reshape(D, 1)

    # all-masked tokens leak a uniform weight (matches softmax of all -1e9)
    uaddv = np.repeat(all_masked.astype(np.float32), K).reshape(1, -1).astype(bf16)
    uadd_chunks = set()
    if all_masked.any():
        for t in np.nonzero(all_masked)[0]:
            core = t // T_CORE
            local_tok = t - core * T_CORE
            uadd_chunks.add(local_tok // TOK_CHUNK)

    key = ("v3", tuple(sorted(uadd_chunks)))
    if key not in _CACHE:
        _CACHE[key] = _build_program(uadd_chunks)
    nc = _CACHE[key]

    shared = {
        "w_kv": w_kv, "w_ka": w_ka, "w_p2ai": w_p2ai,
        "w_p2": w_p2, "w_a2m": w_a2m,
        "w_o": np.ascontiguousarray(Wo), "b_o": b_o,
    }
    in_maps = []
    for c in range(NCORES):
        rs, re = c * R_CORE, (c + 1) * R_CORE
        im = dict(shared)
        im["kf"] = np.ascontiguousarray(kT[:, rs:re].astype(bf16))
        im["rqf"] = np.ascontiguousarray(rqf[:, rs:re])
        im["mrows"] = np.ascontiguousarray(
            mc[rs:re].reshape(NCHUNK, CHUNK).astype(bf16))
        im["uadd"] = np.ascontiguousarray(uaddv[:, rs:re])
        in_maps.append(im)

    res = run_bass_kernel_spmd(nc, in_maps, core_ids=list(range(NCORES)))
    kernel._last_results = res
    out = np.concatenate([res.results[c]["out_f"] for c in range(NCORES)],
                         axis=1)                        # [D, T]
    return np.ascontiguousarray(out.T).reshape(B, N, D).astype(np.float32)
